# revision 1
# baseline (speedup 1.0000x reference)
"""GAT encoder Bass kernel for TRN2 — shared implementation.

Architecture (see notes): dst-sharded nodes across 8 cores; per-core edge-major
"plane-major" layout [128 node-rows, ch-plane, slot]; degree-sorted 128-node
tiles with shared (max-over-core) slot schedule; host ships halo-expanded
source features per slot (x[src] + validity), edge_attr planes, per-node x and
1/deg.  Device computes attention logits with fused scalar_tensor_tensor
cascades (weights-derived scale columns), softmax without max-subtraction
(bounded logits; pad slots carry -100 additive), rank-3 weighted aggregation,
then projects 12->128 (block-diag W_gat), ELU, MLP 128->128 (PReLU) ->32 in
ch-major with PE matmuls.
"""

import numpy as np
import concourse.bass as bass
import concourse.mybir as mybir
import concourse.tile as tile
from concourse.bass import AP

F32 = mybir.dt.float32
AF = mybir.ActivationFunctionType
OP = mybir.AluOpType

P = 128
NEG_SLOPE = 0.2
PAD_V = -100.0


# ---------------------------------------------------------------------------
# Tile-framework epilogue fix: this walrus build rejects >=2 sync waits on the
# kernel-tail Drain ("Too many sync wait commands").  Strip the waits off the
# drain and re-emit them as individual sync-engine nops.
# ---------------------------------------------------------------------------
def patch_tile_epilogue():
    from concourse.tile import ScopedClock
    import bass_rust

    if getattr(tile.TileContext, "_gatk_patched", False):
        return

    # Split multi-wait instructions: this walrus encodes at most one sync wait
    # per instruction.  Excess waits move to same-engine nops just before.
    orig_lower = tile.TileContext._lower_ordered_insts

    def _lower_ordered_insts(self, ordered):
        for bb_name, insts in list(ordered.items()):
            out = []
            for inst in insts:
                si = inst.sync_info
                if si is not None and si.on_wait and len(si.on_wait) > 1:
                    waits = list(si.on_wait)
                    for i, w in enumerate(waits[:-1]):
                        n = bass_rust.InstNoOp(
                            name=f"{inst.name}-sw{i}", ins=[], outs=[])
                        n.engine = inst.engine
                        n.sync_info = mybir.SyncInfo(
                            on_wait=[w], on_update=[])
                        out.append(n)
                    si.on_wait.clear()
                    si.on_wait.append(waits[-1])
                out.append(inst)
            ordered[bb_name] = out
        return orig_lower(self, ordered)

    tile.TileContext._lower_ordered_insts = _lower_ordered_insts
    tile.TileContext._gatk_patched = True

    def _drain_and_barrier(self, tick_clock, wait_clock):
        drain_inst = self.nc.sync.drain()
        wait_clock.add_sem_waits(
            drain_inst.ins, ScopedClock({None: tick_clock.global_clock})
        )
        si = drain_inst.ins.sync_info
        waits = list(si.on_wait or [])
        si.on_wait.clear()
        for w in waits:
            n = self.nc.sync.nop()
            nsi = n.ins.sync_info
            if nsi is None:
                n.ins.sync_info = mybir.SyncInfo(on_wait=[w], on_update=[])
            else:
                nsi.on_wait.append(w)
        self.nc.all_engine_barrier()
        assert self.sems is not None
        popped = self.nc._tile_sem_poison_stack.pop()
        assert popped is self._sem_poison
        self.nc.clear_and_free_semaphores(list(self.sems.allocated().values()))
        self.nc.all_engine_barrier()

    tile.TileContext._drain_and_barrier = _drain_and_barrier


# ---------------------------------------------------------------------------
# Host-side sharding / layout prep (pure indexing + input redistribution).
# ---------------------------------------------------------------------------
def host_prep(x, edge_index, edge_attr, n_cores):
    N = x.shape[0]
    E = edge_index.shape[1]
    NLOC = N // n_cores
    NPAD = ((NLOC + P - 1) // P) * P
    T = NPAD // P

    src = np.asarray(edge_index[0], dtype=np.int64)
    dst = np.asarray(edge_index[1], dtype=np.int64)
    x = np.asarray(x, dtype=np.float32)
    ea = np.asarray(edge_attr, dtype=np.float32)

    deg = np.bincount(dst, minlength=N).astype(np.int64)

    # per-core degree-sorted node order
    orders = np.zeros((n_cores, NPAD), dtype=np.int64)  # sorted-pos -> local id
    ranks = np.zeros((n_cores, NPAD), dtype=np.int64)   # local id -> sorted-pos
    degp = np.zeros((n_cores, NPAD), dtype=np.int64)
    for c in range(n_cores):
        dloc = np.zeros(NPAD, dtype=np.int64)
        dloc[:NLOC] = deg[c * NLOC:(c + 1) * NLOC]
        dloc[NLOC:] = -1  # dummies first
        o = np.argsort(dloc, kind="stable")
        orders[c] = o
        ranks[c, o] = np.arange(NPAD)
        degp[c] = np.maximum(dloc[o], 0)  # sorted-pos -> degree (dummies 0)

    # shared slot schedule
    D = np.zeros(T, dtype=np.int64)
    for t in range(T):
        D[t] = degp[:, t * P:(t + 1) * P].max() + 1
    off = np.zeros(T + 1, dtype=np.int64)
    off[1:] = np.cumsum(D)
    S = int(off[-1])

    # edge -> (core, p, slot)
    e_core = dst // NLOC
    e_rank = ranks[e_core, dst - e_core * NLOC]
    e_t = e_rank // P
    e_p = e_rank % P
    # within-destination running index (1..deg); self-loop is slot 0
    order_e = np.argsort(dst, kind="stable")
    kk = np.empty(E, dtype=np.int64)
    ds = dst[order_e]
    grp_start = np.r_[0, np.flatnonzero(ds[1:] != ds[:-1]) + 1]
    lengths = np.diff(np.r_[grp_start, E])
    within = np.arange(E) - np.repeat(grp_start, lengths)
    kk[order_e] = within + 1
    e_s = off[e_t] + kk

    import ml_dtypes
    bf16 = ml_dtypes.bfloat16
    ea7 = np.zeros((n_cores, P, 7, S), dtype=np.float32)
    xgv = np.zeros((n_cores, P, 4, S), dtype=np.float32)
    xgv[:, :, 3, :] = PAD_V

    ea7[e_core, e_p, :, e_s] = ea
    xgv[e_core, e_p, 0:3, e_s] = x[src]
    xgv[e_core, e_p, 3, e_s] = 0.0

    # self slots + per-node tables
    xn3 = np.zeros((n_cores, P, 3, T), dtype=np.float32)
    invd = np.zeros((n_cores, P, T), dtype=np.float32)
    node_of = np.zeros((n_cores, T, P), dtype=np.int64)
    for c in range(n_cores):
        loc = orders[c]  # sorted-pos -> local id
        glob = c * NLOC + loc
        valid = loc < NLOC
        xg_nodes = np.where(valid[:, None], x[np.minimum(glob, N - 1)], 0.0)
        for t in range(T):
            sl = slice(t * P, (t + 1) * P)
            xn3[c, :, :, t] = xg_nodes[sl]
            xgv[c, :, 0:3, off[t]] = xg_nodes[sl]
            xgv[c, :, 3, off[t]] = 0.0
            invd[c, :, t] = 1.0 / np.maximum(degp[c, sl], 1)
            node_of[c, t] = glob[sl]

    sched = dict(T=T, D=D, off=off, S=S, NLOC=NLOC, NPAD=NPAD, n_cores=n_cores)
    streams = dict(ea7=ea7.astype(bf16), xgv=xgv.astype(bf16), xn3=xn3,
                   invd=invd)
    unscr = dict(node_of=node_of, valid_loc=orders < NLOC)
    return sched, streams, unscr


def host_weights(n_heads, C, W_gat, att_src, att_dst, W_edge, att_edge,
                 bias_gat, W1, b1, prelu_a, W2, b2):
    """Pure-layout reshapes/replications of the weight tensors."""
    HC = n_heads * C
    hmask = np.zeros((P, n_heads), dtype=np.float32)
    for h in range(n_heads):
        hmask[h * C:(h + 1) * C, h] = 1.0
    w = dict(
        w_gat=np.ascontiguousarray(W_gat, dtype=np.float32),          # [3, HC]
        w_gatT=np.ascontiguousarray(W_gat.T, dtype=np.float32),       # [HC, 3]
        w_edgeT=np.ascontiguousarray(W_edge.T, dtype=np.float32),     # [HC, 7]
        att_src_col=np.ascontiguousarray(
            att_src.reshape(HC, 1), dtype=np.float32),
        att_dst_col=np.ascontiguousarray(
            att_dst.reshape(HC, 1), dtype=np.float32),
        att_edge_col=np.ascontiguousarray(
            att_edge.reshape(HC, 1), dtype=np.float32),
        hmask=hmask,
        w1=np.ascontiguousarray(W1, dtype=np.float32),                # [HC, HC]
        w2=np.ascontiguousarray(W2, dtype=np.float32),                # [HC, 32]
        bg_col=np.ascontiguousarray(bias_gat.reshape(HC, 1), dtype=np.float32),
        b1_col=np.ascontiguousarray(b1.reshape(HC, 1), dtype=np.float32),
        pa_col=np.full((HC, 1), float(prelu_a), dtype=np.float32),
        b2rep=np.ascontiguousarray(
            np.broadcast_to(b2.reshape(1, -1), (P, b2.shape[0])),
            dtype=np.float32),
    )
    nj_x = W_gat.shape[0]
    wpj = np.zeros((nj_x * n_heads, HC), dtype=np.float32)
    for h in range(n_heads):
        wpj[nj_x * h: nj_x * (h + 1), C * h: C * (h + 1)] = \
            W_gat[:, C * h: C * (h + 1)]
    w["wpj"] = wpj
    w["ident"] = np.eye(P, dtype=np.float32)
    w["ones_row"] = np.ones((1, P), dtype=np.float32)
    return w


# ---------------------------------------------------------------------------
# Device program.
# ---------------------------------------------------------------------------
def build_program(sched, n_heads=4, nj_x=3, nj_e=7, lat=32, nblocks=2,
                  prelu_alpha=0.25):
    T = sched["T"]
    D = sched["D"]
    off = sched["off"]
    S = sched["S"]
    NPAD = sched["NPAD"]
    HC = P  # hidden dim == 128 == partitions
    H = n_heads

    nc = bass.Bass()
    dt = F32
    BF = mybir.dt.bfloat16

    # --- dram I/O ---
    ea7_d = nc.dram_tensor("ea7", [P, nj_e * S], BF, kind="ExternalInput")
    xgv_d = nc.dram_tensor("xgv", [P, 4 * S], BF, kind="ExternalInput")
    xn3_d = nc.dram_tensor("xn3", [P, nj_x * T], dt, kind="ExternalInput")
    invd_d = nc.dram_tensor("invd", [P, T], dt, kind="ExternalInput")
    wg_d = nc.dram_tensor("w_gat", [nj_x, HC], dt, kind="ExternalInput")
    wgT_d = nc.dram_tensor("w_gatT", [HC, nj_x], dt, kind="ExternalInput")
    weT_d = nc.dram_tensor("w_edgeT", [HC, nj_e], dt, kind="ExternalInput")
    asc_d = nc.dram_tensor("att_src_col", [HC, 1], dt, kind="ExternalInput")
    adc_d = nc.dram_tensor("att_dst_col", [HC, 1], dt, kind="ExternalInput")
    aec_d = nc.dram_tensor("att_edge_col", [HC, 1], dt, kind="ExternalInput")
    hm_d = nc.dram_tensor("hmask", [HC, H], dt, kind="ExternalInput")
    w1_d = nc.dram_tensor("w1", [HC, HC], dt, kind="ExternalInput")
    w2_d = nc.dram_tensor("w2", [HC, lat], dt, kind="ExternalInput")
    bg_d = nc.dram_tensor("bg_col", [HC, 1], dt, kind="ExternalInput")
    b1_d = nc.dram_tensor("b1_col", [HC, 1], dt, kind="ExternalInput")
    pa_d = nc.dram_tensor("pa_col", [HC, 1], dt, kind="ExternalInput")
    b2_d = nc.dram_tensor("b2rep", [P, lat], dt, kind="ExternalInput")
    wpj_d = nc.dram_tensor("wpj", [nj_x * H, HC], dt, kind="ExternalInput")
    id_d = nc.dram_tensor("ident", [P, P], dt, kind="ExternalInput")
    ones_d = nc.dram_tensor("ones_row", [1, P], dt, kind="ExternalInput")
    out_d = nc.dram_tensor("out", [P, T * lat], dt, kind="ExternalOutput")

    NSC = nj_e * H + nj_x * H + nj_x * H  # scale columns: V | U_src | U_dst
    OFF_V, OFF_US, OFF_UD = 0, nj_e * H, nj_e * H + nj_x * H

    # block split of tiles
    bl = []
    tpb = (T + nblocks - 1) // nblocks
    for b in range(nblocks):
        t0, t1 = b * tpb, min((b + 1) * tpb, T)
        if t0 < t1:
            bl.append((t0, t1))

    Dmax = int(D.max())
    NCH = T * lat // P + (1 if (T * lat) % P else 0)

    with tile.TileContext(nc) as tc:
        with (
            tc.tile_pool(name="wp", bufs=1) as wp,
            tc.tile_pool(name="sp", bufs=2) as sp,
            tc.tile_pool(name="mp", bufs=2) as mp,
            tc.tile_pool(name="pp", bufs=1, space="PSUM") as pp,
            tc.tile_pool(name="pq", bufs=1, space="PSUM") as pq,
        ):
            # ---------------- phase 0: weights & derived ----------------
            wgat = wp.tile([nj_x, HC], dt, tag="wgat")
            wgT = wp.tile([HC, nj_x], dt, tag="wgT")
            weT = wp.tile([HC, nj_e], dt, tag="weT")
            asc = wp.tile([HC, 1], dt, tag="asc")
            adc = wp.tile([HC, 1], dt, tag="adc")
            aec = wp.tile([HC, 1], dt, tag="aec")
            hma = wp.tile([HC, H], dt, tag="hma")
            w1s = wp.tile([HC, HC], dt, tag="w1s")
            w2s = wp.tile([HC, lat], dt, tag="w2s")
            bgc = wp.tile([HC, 1], dt, tag="bgc")
            b1c = wp.tile([HC, 1], dt, tag="b1c")
            pac = wp.tile([HC, 1], dt, tag="pac")
            b2r = wp.tile([P, lat], dt, tag="b2r")
            xns = wp.tile([P, nj_x * T], dt, tag="xns")
            ivd = wp.tile([P, T], dt, tag="ivd")
            for dst_t, src_t in [
                (wgat, wg_d), (wgT, wgT_d), (weT, weT_d), (asc, asc_d),
                (adc, adc_d), (aec, aec_d), (hma, hm_d), (w1s, w1_d),
                (w2s, w2_d), (bgc, bg_d), (b1c, b1_d), (pac, pa_d),
                (b2r, b2_d), (xns, xn3_d), (ivd, invd_d),
            ]:
                nc.sync.dma_start(dst_t[:], src_t[:])

            ident = wp.tile([P, P], dt, tag="ident")
            nc.sync.dma_start(ident[:], id_d[:])
            onesr = wp.tile([1, P], dt, tag="onesr")
            nc.sync.dma_start(onesr[:], ones_d[:])

            # W28 = W_edgeT (j-major x H) * head-mask ; W12 same from W_gatT
            w28 = wp.tile([HC, nj_e * H], dt, tag="w28")
            w12 = wp.tile([HC, nj_x * H], dt, tag="w12")
            weT_b = AP(weT[:].tensor, weT[:].offset,
                       [list(weT[:].ap[0]), [1, nj_e], [0, H]])
            hm_e = AP(hma[:].tensor, hma[:].offset,
                      [list(hma[:].ap[0]), [0, nj_e], [1, H]])
            nc.vector.tensor_tensor(
                out=w28[:].rearrange("p (j h) -> p j h", j=nj_e),
                in0=weT_b, in1=hm_e, op=OP.mult)
            wgT_b = AP(wgT[:].tensor, wgT[:].offset,
                       [list(wgT[:].ap[0]), [1, nj_x], [0, H]])
            hm_x = AP(hma[:].tensor, hma[:].offset,
                      [list(hma[:].ap[0]), [0, nj_x], [1, H]])
            nc.vector.tensor_tensor(
                out=w12[:].rearrange("p (j h) -> p j h", j=nj_x),
                in0=wgT_b, in1=hm_x, op=OP.mult)

            # scale rows via K=128 matmuls, then partition-broadcast
            srow = wp.tile([1, NSC], dt, tag="srow")
            psv = pq.tile([1, NSC], dt, tag="pst")
            nc.tensor.matmul(psv[:, 0:nj_e * H], aec[:], w28[:],
                             start=True, stop=True)
            nc.tensor.matmul(psv[:, OFF_US:OFF_US + nj_x * H], asc[:], w12[:],
                             start=True, stop=True)
            nc.tensor.matmul(psv[:, OFF_UD:OFF_UD + nj_x * H], adc[:], w12[:],
                             start=True, stop=True)
            nc.vector.tensor_copy(srow[:], psv[:])
            scal = wp.tile([P, NSC], dt, tag="scal")
            psb = pq.tile([P, NSC], dt, tag="pso")
            nc.tensor.matmul(psb[:], onesr[:], srow[:], start=True, stop=True)
            nc.vector.tensor_copy(scal[:], psb[:])

            # ad_all [P, H, T] from xn planes
            ad_all = wp.tile([P, H * T], dt, tag="ad_all")
            for h in range(H):
                adh = ad_all[:, h * T:(h + 1) * T]
                nc.vector.tensor_scalar(
                    out=adh, in0=xns[:, 0:T],
                    scalar1=scal[:, OFF_UD + 0 * H + h: OFF_UD + 0 * H + h + 1],
                    scalar2=None, op0=OP.mult)
                for j in range(1, nj_x):
                    nc.vector.scalar_tensor_tensor(
                        out=adh, in0=xns[:, j * T:(j + 1) * T],
                        scalar=scal[:, OFF_UD + j * H + h: OFF_UD + j * H + h + 1],
                        in1=adh, op0=OP.mult, op1=OP.add)

            # Wp [12, 128] block-diagonal (host-assembled pure layout)
            wpj = wp.tile([nj_x * H, HC], dt, tag="wpj")
            nc.sync.dma_start(wpj[:], wpj_d[:])



            # derived prelu columns
            ompa = wp.tile([HC, 1], dt, tag="ompa")
            ab1 = wp.tile([HC, 1], dt, tag="ab1")
            nc.vector.tensor_scalar(out=ompa[:], in0=pac[:], scalar1=-1.0,
                                    scalar2=1.0, op0=OP.mult, op1=OP.add)
            nc.vector.tensor_tensor(out=ab1[:], in0=pac[:], in1=b1c[:],
                                    op=OP.mult)

            # persistent accumulators
            den_all = wp.tile([P, H * T], dt, tag="den_all")
            agg_all = wp.tile([P, nj_x * H * T], dt, tag="agg_all")
            rec_all = wp.tile([P, H * T], dt, tag="rec_all")
            out_sb = wp.tile([P, T * lat], dt, tag="out_sb")

            # ---------------- phase 1: per-block edge pipeline ----------------
            for (t0, t1) in bl:
                o0, o1 = int(off[t0]), int(off[t1])
                SB = o1 - o0
                eab = sp.tile([P, nj_e * SB], BF, tag="eab")
                xgb = sp.tile([P, 4 * SB], BF, tag="xgb")
                aeb = sp.tile([P, H * SB], dt, tag="aeb")
                exb = sp.tile([P, H * SB], BF, tag="exb")

                # strided DMA loads of the block's plane slices
                nc.sync.dma_start(
                    eab[:].rearrange("p (j s) -> p j s", j=nj_e),
                    ea7_d[:].rearrange("p (j s) -> p j s", j=nj_e)[:, :, o0:o1])
                nc.sync.dma_start(
                    xgb[:].rearrange("p (j s) -> p j s", j=4),
                    xgv_d[:].rearrange("p (j s) -> p j s", j=4)[:, :, o0:o1])

                # cascade B: ae = sum_j ea_j * V[j,h]
                for h in range(H):
                    aeh = aeb[:, h * SB:(h + 1) * SB]
                    nc.vector.tensor_scalar(
                        out=aeh, in0=eab[:, 0:SB],
                        scalar1=scal[:, OFF_V + 0 * H + h: OFF_V + 0 * H + h + 1],
                        scalar2=None, op0=OP.mult)
                    for j in range(1, nj_e):
                        nc.vector.scalar_tensor_tensor(
                            out=aeh, in0=eab[:, j * SB:(j + 1) * SB],
                            scalar=scal[:, OFF_V + j * H + h: OFF_V + j * H + h + 1],
                            in1=aeh, op0=OP.mult, op1=OP.add)

                ae_t = aeb[:].tensor
                ae_o = aeb[:].offset
                ae_p = list(aeb[:].ap[0])

                # self-loop ae = mean of real ae (per tile)
                for t in range(t0, t1):
                    lt = int(off[t]) - o0
                    dt_t = int(D[t])
                    aes = mp.tile([P, H], dt, tag="aes")
                    nc.vector.tensor_reduce(
                        out=aes[:],
                        in_=AP(ae_t, ae_o + lt, [ae_p, [SB, H], [1, dt_t]]),
                        axis=mybir.AxisListType.X, op=OP.add)
                    nc.vector.tensor_scalar(
                        out=AP(ae_t, ae_o + lt, [ae_p, [SB, H]]), in0=aes[:],
                        scalar1=ivd[:, t:t + 1], scalar2=None, op0=OP.mult)

                # cascade D: += x[src]-derived a_src + validity
                for h in range(H):
                    aeh = aeb[:, h * SB:(h + 1) * SB]
                    for j in range(nj_x):
                        nc.vector.scalar_tensor_tensor(
                            out=aeh, in0=xgb[:, j * SB:(j + 1) * SB],
                            scalar=scal[:, OFF_US + j * H + h: OFF_US + j * H + h + 1],
                            in1=aeh, op0=OP.mult, op1=OP.add)
                    nc.vector.scalar_tensor_tensor(
                        out=aeh, in0=xgb[:, 3 * SB:4 * SB], scalar=1.0,
                        in1=aeh, op0=OP.mult, op1=OP.add)

                # += a_dst (per tile broadcast)
                for t in range(t0, t1):
                    lt = int(off[t]) - o0
                    dt_t = int(D[t])
                    sl = AP(ae_t, ae_o + lt, [ae_p, [SB, H], [1, dt_t]])
                    adb = AP(ad_all[:].tensor, ad_all[:].offset + t,
                             [list(ad_all[:].ap[0]), [T, H], [0, dt_t]])
                    nc.vector.tensor_tensor(out=sl, in0=sl, in1=adb, op=OP.add)

                # leaky relu (ACT Prelu) then exp
                nc.scalar.activation(aeb[:], aeb[:], AF.Prelu, alpha=NEG_SLOPE)
                nc.scalar.activation(exb[:], aeb[:], AF.Exp)

                # denominators + weighted aggregation
                ex_t = exb[:].tensor
                ex_o = exb[:].offset
                ex_p = list(exb[:].ap[0])
                for t in range(t0, t1):
                    lt = int(off[t]) - o0
                    dt_t = int(D[t])
                    nc.vector.tensor_reduce(
                        out=AP(den_all[:].tensor, den_all[:].offset + t,
                               [list(den_all[:].ap[0]), [T, H]]),
                        in_=AP(ex_t, ex_o + lt, [ex_p, [SB, H], [1, dt_t]]),
                        axis=mybir.AxisListType.X, op=OP.add)

                    msg = mp.tile([P, H * nj_x * Dmax], BF, tag="msg")
                    m_ap = AP(msg[:].tensor, msg[:].offset,
                              [list(msg[:].ap[0]), [nj_x * dt_t, H],
                               [dt_t, nj_x], [1, dt_t]])
                    ealpha = AP(ex_t, ex_o + lt,
                                [ex_p, [SB, H], [0, nj_x], [1, dt_t]])
                    xgs = AP(xgb[:].tensor, xgb[:].offset + lt,
                             [list(xgb[:].ap[0]), [0, H], [SB, nj_x], [1, dt_t]])
                    nc.vector.tensor_tensor(out=m_ap, in0=ealpha, in1=xgs,
                                            op=OP.mult)
                    nc.vector.tensor_reduce(
                        out=AP(agg_all[:].tensor, agg_all[:].offset + t,
                               [list(agg_all[:].ap[0]), [T, H * nj_x]]),
                        in_=AP(msg[:].tensor, msg[:].offset,
                               [list(msg[:].ap[0]), [dt_t, H * nj_x],
                                [1, dt_t]]),
                        axis=mybir.AxisListType.X, op=OP.add)

            # ---------------- phase 2: normalize + project + MLP ----------------
            nc.vector.reciprocal(rec_all[:], den_all[:])
            agg_b = AP(agg_all[:].tensor, agg_all[:].offset,
                       [list(agg_all[:].ap[0]), [nj_x * T, H], [T, nj_x], [1, T]])
            rec_b = AP(rec_all[:].tensor, rec_all[:].offset,
                       [list(rec_all[:].ap[0]), [T, H], [0, nj_x], [1, T]])
            nc.vector.tensor_tensor(out=agg_b, in0=agg_b, in1=rec_b, op=OP.mult)

            # chunked: transpose agg -> matmul chain -> transpose out
            n_chunks = (T + 3) // 4
            for cch in range(n_chunks):
                ta, tb = cch * 4, min(cch * 4 + 4, T)
                cw = (tb - ta) * P

                pst = pq.tile([nj_x * H, cw], dt, tag="pst")
                for ti in range(ta, tb):
                    nc.tensor.transpose(
                        out=pst[:, (ti - ta) * P:(ti - ta + 1) * P],
                        in_=AP(agg_all[:].tensor, agg_all[:].offset + ti,
                               [list(agg_all[:].ap[0]), [T, nj_x * H]]),
                        identity=ident[:])
                aggT = mp.tile([nj_x * H, 4 * P], dt, tag="aggT")
                nc.scalar.copy(aggT[:, :cw], pst[:])

                ps1 = pp.tile([HC, cw], dt, tag="ps1")
                nc.tensor.matmul(ps1[:], wpj[:], aggT[:, :cw],
                                 start=True, stop=True)
                # ELU(z+bg): min(exp(z+bg),1) - 1 + relu(z+bg)
                r1 = mp.tile([HC, 4 * P], dt, tag="r1")
                u1 = mp.tile([HC, 4 * P], dt, tag="u1")
                nc.scalar.activation(r1[:, :cw], ps1[:], AF.Relu, bias=bgc[:, :1])
                nc.scalar.activation(u1[:, :cw], ps1[:], AF.Exp, bias=bgc[:, :1])
                nc.vector.tensor_scalar(out=u1[:, :cw], in0=u1[:, :cw],
                                        scalar1=1.0, scalar2=None, op0=OP.min)
                h1 = mp.tile([HC, 4 * P], dt, tag="h1")
                nc.vector.scalar_tensor_tensor(
                    out=h1[:, :cw], in0=u1[:, :cw], scalar=-1.0, in1=r1[:, :cw],
                    op0=OP.add, op1=OP.add)

                ps2 = pp.tile([HC, cw], dt, tag="ps2")
                nc.tensor.matmul(ps2[:], w1s[:], h1[:, :cw],
                                 start=True, stop=True)
                h2 = mp.tile([HC, 4 * P], dt, tag="h2")
                nc.scalar.activation(h2[:, :cw], ps2[:], AF.Prelu,
                                     bias=b1c[:, :1], alpha=prelu_alpha)

                ps3 = pp.tile([lat, cw], dt, tag="ps3")
                nc.tensor.matmul(ps3[:], w2s[:], h2[:, :cw],
                                 start=True, stop=True)
                o3 = mp.tile([lat, 4 * P], dt, tag="o3")
                nc.scalar.copy(o3[:, :cw], ps3[:])

                pso = pq.tile([P, 4 * lat], dt, tag="pso")
                for ti in range(ta, tb):
                    nc.tensor.transpose(
                        out=pso[:, (ti - ta) * lat:(ti - ta + 1) * lat],
                        in_=o3[:, (ti - ta) * P:(ti - ta + 1) * P],
                        identity=ident[:lat, :lat])
                b2b = AP(b2r[:].tensor, b2r[:].offset,
                         [list(b2r[:].ap[0]), [0, tb - ta], [1, lat]])
                nc.vector.scalar_tensor_tensor(
                    out=out_sb[:, ta * lat: tb * lat], in0=pso[:, :(tb - ta) * lat],
                    scalar=1.0, in1=b2b, op0=OP.mult, op1=OP.add)

            nc.sync.dma_start(out_d[:], out_sb[:])

    return nc


# ---------------------------------------------------------------------------
# Full kernel entry (host orchestration).
# ---------------------------------------------------------------------------
def make_in_maps(sched, streams, w, n_cores):
    maps = []
    for c in range(n_cores):
        m = dict(
            ea7=streams["ea7"][c].reshape(P, -1),
            xgv=streams["xgv"][c].reshape(P, -1),
            xn3=streams["xn3"][c].reshape(P, -1),
            invd=streams["invd"][c],
        )
        m.update(w)
        # rename to dram tensor names
        m = {
            "ea7": m["ea7"], "xgv": m["xgv"], "xn3": m["xn3"],
            "invd": m["invd"], "w_gat": w["w_gat"], "w_gatT": w["w_gatT"],
            "w_edgeT": w["w_edgeT"], "att_src_col": w["att_src_col"],
            "att_dst_col": w["att_dst_col"], "att_edge_col": w["att_edge_col"],
            "hmask": w["hmask"], "w1": w["w1"], "w2": w["w2"],
            "bg_col": w["bg_col"], "b1_col": w["b1_col"], "pa_col": w["pa_col"],
            "b2rep": w["b2rep"], "wpj": w["wpj"],
            "ident": w["ident"], "ones_row": w["ones_row"],
        }
        maps.append(m)
    return maps


def unscramble(results, sched, unscr, N, lat=32):
    n_cores = sched["n_cores"]
    T = sched["T"]
    NLOC = sched["NLOC"]
    out = np.zeros((N, lat), dtype=np.float32)
    for c in range(n_cores):
        o = results[c]["out"].reshape(P, T, lat)
        node_of = unscr["node_of"][c]  # [T, P] global ids (clamped for dummies)
        valid = unscr["valid_loc"][c].reshape(T, P)
        for t in range(T):
            v = valid[t]
            out[node_of[t][v]] = o[v, t]
    return out


# ---------------------------------------------------------------------------
# Self-contained harness entry: kernel(**inputs) -> full [N, 32] output.
# ---------------------------------------------------------------------------
_CACHE = {}


def kernel(x, edge_index, edge_attr, W_gat, att_src, att_dst, W_edge,
           att_edge, bias_gat, W1, b1, prelu_a, W2, b2):
    from concourse.bass_utils import run_bass_kernel_spmd

    patch_tile_epilogue()
    n_cores = 8
    x = np.asarray(x)
    edge_index = np.asarray(edge_index)
    edge_attr = np.asarray(edge_attr)
    H, C = np.asarray(att_src).shape

    sched, streams, unscr = host_prep(x, edge_index, edge_attr, n_cores)
    w = host_weights(H, C, np.asarray(W_gat), np.asarray(att_src),
                     np.asarray(att_dst), np.asarray(W_edge),
                     np.asarray(att_edge), np.asarray(bias_gat),
                     np.asarray(W1), np.asarray(b1), np.asarray(prelu_a),
                     np.asarray(W2), np.asarray(b2))

    key = (sched["T"], sched["S"], tuple(int(d) for d in sched["D"]),
           float(np.asarray(prelu_a)))
    if key not in _CACHE:
        _CACHE[key] = build_program(sched, n_heads=H, nblocks=2,
                                    prelu_alpha=float(np.asarray(prelu_a)))
    nc = _CACHE[key]

    maps = make_in_maps(sched, streams, w, n_cores)
    res = run_bass_kernel_spmd(nc, maps, core_ids=list(range(n_cores)))
    out = unscramble(res.results, sched, unscr, x.shape[0])
    return out.astype(np.float32)



# revision 13
# speedup vs baseline: 1.3324x; 1.3324x over previous
"""GAT encoder Bass kernel for TRN2 — v2.

Architecture: dst-sharded nodes across 8 cores; per-core edge-major
"plane-major" layout [128 node-rows, ch-plane, slot]; degree-sorted 128-node
tiles with shared (max-over-core) slot schedule, slot count UNIFORM within
each 4-tile chunk so per-tile ops batch into one instruction per chunk.
Host ships fp16 halo-expanded source features per slot (x[src]), fp16
edge_attr planes, per-node x, 1/deg, and pad counts.

Device: attention logits via tensor_scalar leaves (4x DVE mode) + fp16
tensor_tensor merge trees; self-loop logit = mean of real edge logits
(per-chunk batched reduce); a_dst broadcast add on GpSimd; leaky-relu+exp on
ACT; per-chunk denominator/aggregation reduces on DVE with a pad-slot
denominator correction (no validity plane); projection 12->128 (block-diag
W_gat fp16), ELU, MLP 128->128 (PReLU) ->32 in ch-major with fp16 PE
matmuls, double-buffered PSUM, per-chunk output DMA.
"""

import numpy as np
import concourse.bass as bass
import concourse.mybir as mybir
import concourse.tile as tile
from concourse.bass import AP

F32 = mybir.dt.float32
F16 = mybir.dt.float16
AF = mybir.ActivationFunctionType
OP = mybir.AluOpType

P = 128
NEG_SLOPE = 0.2
CH = 4  # tiles per chunk (uniform slot count within a chunk)
MSHIFT = -8.0  # logit shift before exp (cancels in softmax; avoids fp16 inf)


# ---------------------------------------------------------------------------
# Tile-framework epilogue fix: this walrus build rejects >=2 sync waits on the
# kernel-tail Drain ("Too many sync wait commands").  Strip the waits off the
# drain and re-emit them as individual sync-engine nops.
# ---------------------------------------------------------------------------
def patch_tile_epilogue():
    from concourse.tile import ScopedClock
    import bass_rust

    if getattr(tile.TileContext, "_gatk_patched", False):
        return

    orig_lower = tile.TileContext._lower_ordered_insts

    def _lower_ordered_insts(self, ordered):
        for bb_name, insts in list(ordered.items()):
            out = []
            for inst in insts:
                si = inst.sync_info
                if si is not None and si.on_wait and len(si.on_wait) > 1:
                    waits = list(si.on_wait)
                    for i, w in enumerate(waits[:-1]):
                        n = bass_rust.InstNoOp(
                            name=f"{inst.name}-sw{i}", ins=[], outs=[])
                        n.engine = inst.engine
                        n.sync_info = mybir.SyncInfo(
                            on_wait=[w], on_update=[])
                        out.append(n)
                    si.on_wait.clear()
                    si.on_wait.append(waits[-1])
                out.append(inst)
            ordered[bb_name] = out
        return orig_lower(self, ordered)

    tile.TileContext._lower_ordered_insts = _lower_ordered_insts
    tile.TileContext._gatk_patched = True

    def _drain_and_barrier(self, tick_clock, wait_clock):
        drain_inst = self.nc.sync.drain()
        wait_clock.add_sem_waits(
            drain_inst.ins, ScopedClock({None: tick_clock.global_clock})
        )
        si = drain_inst.ins.sync_info
        waits = list(si.on_wait or [])
        si.on_wait.clear()
        for w in waits:
            n = self.nc.sync.nop()
            nsi = n.ins.sync_info
            if nsi is None:
                n.ins.sync_info = mybir.SyncInfo(on_wait=[w], on_update=[])
            else:
                nsi.on_wait.append(w)
        self.nc.all_engine_barrier()
        assert self.sems is not None
        popped = self.nc._tile_sem_poison_stack.pop()
        assert popped is self._sem_poison
        self.nc.clear_and_free_semaphores(list(self.sems.allocated().values()))
        self.nc.all_engine_barrier()

    tile.TileContext._drain_and_barrier = _drain_and_barrier


# ---------------------------------------------------------------------------
# Host-side sharding / layout prep (pure indexing + input redistribution).
# ---------------------------------------------------------------------------
def host_prep(x, edge_index, edge_attr, n_cores):
    N = x.shape[0]
    E = edge_index.shape[1]
    NLOC = N // n_cores
    NPAD = ((NLOC + P - 1) // P) * P
    T = NPAD // P

    src = np.asarray(edge_index[0], dtype=np.int64)
    dst = np.asarray(edge_index[1], dtype=np.int64)
    x = np.asarray(x, dtype=np.float32)
    ea = np.asarray(edge_attr, dtype=np.float32)

    deg = np.bincount(dst, minlength=N).astype(np.int64)

    # per-core degree-sorted node order
    orders = np.zeros((n_cores, NPAD), dtype=np.int64)  # sorted-pos -> local id
    ranks = np.zeros((n_cores, NPAD), dtype=np.int64)   # local id -> sorted-pos
    degp = np.zeros((n_cores, NPAD), dtype=np.int64)
    for c in range(n_cores):
        dloc = np.zeros(NPAD, dtype=np.int64)
        dloc[:NLOC] = deg[c * NLOC:(c + 1) * NLOC]
        dloc[NLOC:] = -1  # dummies first
        o = np.argsort(dloc, kind="stable")
        orders[c] = o
        ranks[c, o] = np.arange(NPAD)
        degp[c] = np.maximum(dloc[o], 0)  # sorted-pos -> degree (dummies 0)

    # shared slot schedule; D uniform within each CH-tile chunk
    D = np.zeros(T, dtype=np.int64)
    for t in range(T):
        D[t] = degp[:, t * P:(t + 1) * P].max() + 1
    for g in range((T + CH - 1) // CH):
        t0, t1 = g * CH, min((g + 1) * CH, T)
        D[t0:t1] = D[t0:t1].max()
    off = np.zeros(T + 1, dtype=np.int64)
    off[1:] = np.cumsum(D)
    S = int(off[-1])

    # edge -> (core, p, slot)
    e_core = dst // NLOC
    e_rank = ranks[e_core, dst - e_core * NLOC]
    e_t = e_rank // P
    e_p = e_rank % P
    # within-destination running index (1..deg); self-loop is slot 0
    order_e = np.argsort(dst, kind="stable")
    kk = np.empty(E, dtype=np.int64)
    ds = dst[order_e]
    grp_start = np.r_[0, np.flatnonzero(ds[1:] != ds[:-1]) + 1]
    lengths = np.diff(np.r_[grp_start, E])
    within = np.arange(E) - np.repeat(grp_start, lengths)
    kk[order_e] = within + 1
    e_s = off[e_t] + kk

    ea7 = np.zeros((n_cores, P, 7, S), dtype=np.float32)
    xgv = np.zeros((n_cores, P, 3, S), dtype=np.float32)

    ea7[e_core, e_p, :, e_s] = ea
    xgv[e_core, e_p, :, e_s] = x[src]

    # self slots + per-node tables
    xn3 = np.zeros((n_cores, P, 3, T), dtype=np.float32)
    invd = np.zeros((n_cores, P, T), dtype=np.float32)
    npad = np.zeros((n_cores, P, T), dtype=np.float32)
    node_of = np.zeros((n_cores, T, P), dtype=np.int64)
    for c in range(n_cores):
        loc = orders[c]  # sorted-pos -> local id
        glob = c * NLOC + loc
        valid = loc < NLOC
        xg_nodes = np.where(valid[:, None], x[np.minimum(glob, N - 1)], 0.0)
        for t in range(T):
            sl = slice(t * P, (t + 1) * P)
            xn3[c, :, :, t] = xg_nodes[sl]
            xgv[c, :, :, off[t]] = xg_nodes[sl]
            invd[c, :, t] = 1.0 / np.maximum(degp[c, sl], 1)
            npad[c, :, t] = (D[t] - 1) - degp[c, sl]
            node_of[c, t] = glob[sl]

    sched = dict(T=T, D=D, off=off, S=S, NLOC=NLOC, NPAD=NPAD, n_cores=n_cores)
    streams = dict(ea7=ea7.astype(np.float16), xgv=xgv.astype(np.float16),
                   xn3=xn3, invd=invd, npad=npad)
    unscr = dict(node_of=node_of, valid_loc=orders < NLOC)
    return sched, streams, unscr


def host_weights(n_heads, C, W_gat, att_src, att_dst, W_edge, att_edge,
                 bias_gat, W1, b1, prelu_a, W2, b2):
    """Pure-layout reshapes/replications/casts of the weight tensors."""
    HC = n_heads * C
    hmask = np.zeros((P, n_heads), dtype=np.float32)
    for h in range(n_heads):
        hmask[h * C:(h + 1) * C, h] = 1.0
    nj_x = W_gat.shape[0]
    wpj = np.zeros((nj_x * n_heads, HC), dtype=np.float32)
    for h in range(n_heads):
        wpj[nj_x * h: nj_x * (h + 1), C * h: C * (h + 1)] = \
            W_gat[:, C * h: C * (h + 1)]
    w = dict(
        w_gatT=np.ascontiguousarray(W_gat.T, dtype=np.float32),       # [HC, 3]
        w_edgeT=np.ascontiguousarray(W_edge.T, dtype=np.float32),     # [HC, 7]
        att_src_col=np.ascontiguousarray(
            att_src.reshape(HC, 1), dtype=np.float32),
        att_dst_col=np.ascontiguousarray(
            att_dst.reshape(HC, 1), dtype=np.float32),
        att_edge_col=np.ascontiguousarray(
            att_edge.reshape(HC, 1), dtype=np.float32),
        hmask=hmask,
        wpj16=np.ascontiguousarray(wpj, dtype=np.float16),
        w116=np.ascontiguousarray(W1, dtype=np.float16),              # [HC, HC]
        w216=np.ascontiguousarray(W2, dtype=np.float16),              # [HC, 32]
        bg_col=np.ascontiguousarray(bias_gat.reshape(HC, 1), dtype=np.float32),
        b1_col=np.ascontiguousarray(b1.reshape(HC, 1), dtype=np.float32),
        b2rep=np.ascontiguousarray(
            np.broadcast_to(b2.reshape(1, -1), (P, b2.shape[0])),
            dtype=np.float32),
        ident=np.eye(P, dtype=np.float32),
        ones_row=np.ones((1, P), dtype=np.float32),
    )
    return w


# ---------------------------------------------------------------------------
# Device program.
# ---------------------------------------------------------------------------
def build_program(sched, n_heads=4, nj_x=3, nj_e=7, lat=32,
                  prelu_alpha=0.25):
    T = sched["T"]
    D = sched["D"]
    off = sched["off"]
    S = sched["S"]
    HC = P  # hidden dim == 128 == partitions
    H = n_heads
    NG = (T + CH - 1) // CH  # chunk groups

    nc = bass.Bass()
    dt = F32

    # --- dram I/O ---
    ea7_d = nc.dram_tensor("ea7", [P, nj_e * S], F16, kind="ExternalInput")
    xgv_d = nc.dram_tensor("xgv", [P, nj_x * S], F16, kind="ExternalInput")
    xn3_d = nc.dram_tensor("xn3", [P, nj_x * T], dt, kind="ExternalInput")
    invd_d = nc.dram_tensor("invd", [P, T], dt, kind="ExternalInput")
    npad_d = nc.dram_tensor("npad", [P, T], dt, kind="ExternalInput")
    wgT_d = nc.dram_tensor("w_gatT", [HC, nj_x], dt, kind="ExternalInput")
    weT_d = nc.dram_tensor("w_edgeT", [HC, nj_e], dt, kind="ExternalInput")
    asc_d = nc.dram_tensor("att_src_col", [HC, 1], dt, kind="ExternalInput")
    adc_d = nc.dram_tensor("att_dst_col", [HC, 1], dt, kind="ExternalInput")
    aec_d = nc.dram_tensor("att_edge_col", [HC, 1], dt, kind="ExternalInput")
    hm_d = nc.dram_tensor("hmask", [HC, H], dt, kind="ExternalInput")
    wpj16_d = nc.dram_tensor("wpj16", [nj_x * H, HC], F16, kind="ExternalInput")
    w116_d = nc.dram_tensor("w116", [HC, HC], F16, kind="ExternalInput")
    w216_d = nc.dram_tensor("w216", [HC, lat], F16, kind="ExternalInput")
    bg_d = nc.dram_tensor("bg_col", [HC, 1], dt, kind="ExternalInput")
    b1_d = nc.dram_tensor("b1_col", [HC, 1], dt, kind="ExternalInput")
    b2_d = nc.dram_tensor("b2rep", [P, lat], dt, kind="ExternalInput")
    id_d = nc.dram_tensor("ident", [P, P], dt, kind="ExternalInput")
    ones_d = nc.dram_tensor("ones_row", [1, P], dt, kind="ExternalInput")
    out_d = nc.dram_tensor("out", [P, T * lat], dt, kind="ExternalOutput")

    NSC = nj_e * H + nj_x * H + nj_x * H  # scale columns: V | U_src | U_dst
    OFF_V, OFF_US, OFF_UD = 0, nj_e * H, nj_e * H + nj_x * H

    # two blocks split at a chunk boundary
    tmid = (T // 2 // CH) * CH
    bl = [(0, tmid), (tmid, T)]
    SBmax = max(int(off[t1] - off[t0]) for (t0, t1) in bl)
    Dmax = int(D.max())
    CW = CH * P  # phase-2 chunk column width

    with tile.TileContext(nc) as tc:
        with (
            tc.tile_pool(name="wp", bufs=1) as wp,
            tc.tile_pool(name="sp", bufs=2) as sp,
            tc.tile_pool(name="mp", bufs=2) as mp,
            tc.tile_pool(name="pp", bufs=2, space="PSUM") as pp,
            tc.tile_pool(name="pq", bufs=1, space="PSUM") as pq,
        ):
            # ---------------- phase 0: weights & derived ----------------
            wgT = wp.tile([HC, nj_x], dt, tag="wgT")
            weT = wp.tile([HC, nj_e], dt, tag="weT")
            asc = wp.tile([HC, 1], dt, tag="asc")
            adc = wp.tile([HC, 1], dt, tag="adc")
            aec = wp.tile([HC, 1], dt, tag="aec")
            hma = wp.tile([HC, H], dt, tag="hma")
            wpj = wp.tile([nj_x * H, HC], F16, tag="wpj")
            w1s = wp.tile([HC, HC], F16, tag="w1s")
            w2s = wp.tile([HC, lat], F16, tag="w2s")
            bgc = wp.tile([HC, 1], dt, tag="bgc")
            b1c = wp.tile([HC, 1], dt, tag="b1c")
            b2r = wp.tile([P, lat], dt, tag="b2r")
            xns = wp.tile([P, nj_x * T], dt, tag="xns")
            ivd = wp.tile([P, T], dt, tag="ivd")
            npd = wp.tile([P, T], dt, tag="npd")
            ident = wp.tile([P, P], dt, tag="ident")
            onesr = wp.tile([1, P], dt, tag="onesr")
            for dst_t, src_t in [
                (wgT, wgT_d), (weT, weT_d), (asc, asc_d),
                (adc, adc_d), (aec, aec_d), (hma, hm_d), (wpj, wpj16_d),
                (w1s, w116_d), (w2s, w216_d), (bgc, bg_d), (b1c, b1_d),
                (b2r, b2_d), (xns, xn3_d), (ivd, invd_d), (npd, npad_d),
                (ident, id_d), (onesr, ones_d),
            ]:
                nc.sync.dma_start(dst_t[:], src_t[:])

            # W28 = W_edgeT (j-major x H) * head-mask ; W12 same from W_gatT
            w28 = wp.tile([HC, nj_e * H], dt, tag="w28")
            w12 = wp.tile([HC, nj_x * H], dt, tag="w12")
            weT_b = AP(weT[:].tensor, weT[:].offset,
                       [list(weT[:].ap[0]), [1, nj_e], [0, H]])
            hm_e = AP(hma[:].tensor, hma[:].offset,
                      [list(hma[:].ap[0]), [0, nj_e], [1, H]])
            nc.vector.tensor_tensor(
                out=w28[:].rearrange("p (j h) -> p j h", j=nj_e),
                in0=weT_b, in1=hm_e, op=OP.mult)
            wgT_b = AP(wgT[:].tensor, wgT[:].offset,
                       [list(wgT[:].ap[0]), [1, nj_x], [0, H]])
            hm_x = AP(hma[:].tensor, hma[:].offset,
                      [list(hma[:].ap[0]), [0, nj_x], [1, H]])
            nc.vector.tensor_tensor(
                out=w12[:].rearrange("p (j h) -> p j h", j=nj_x),
                in0=wgT_b, in1=hm_x, op=OP.mult)

            # scale rows via K=128 matmuls, then partition-broadcast
            ps1w = pp.tile([HC, CW], dt, tag="ps1")
            ps2w = pp.tile([HC, CW], dt, tag="ps2")
            srow = wp.tile([1, NSC], dt, tag="srow")
            psv = ps1w[0:1, 0:NSC]
            nc.tensor.matmul(psv[:, 0:nj_e * H], aec[:], w28[:],
                             start=True, stop=True)
            nc.tensor.matmul(psv[:, OFF_US:OFF_US + nj_x * H], asc[:], w12[:],
                             start=True, stop=True)
            nc.tensor.matmul(psv[:, OFF_UD:OFF_UD + nj_x * H], adc[:], w12[:],
                             start=True, stop=True)
            nc.vector.tensor_copy(srow[:], psv)
            scal = wp.tile([P, NSC], dt, tag="scal")
            psb = ps2w[0:P, 0:NSC]
            nc.tensor.matmul(psb, onesr[:], srow[:], start=True, stop=True)
            nc.vector.tensor_copy(scal[:], psb)

            # ad_all [P, H, T] from xn planes (a_dst per node)
            ad_all = wp.tile([P, H * T], dt, tag="ad_all")
            for h in range(H):
                adh = ad_all[:, h * T:(h + 1) * T]
                nc.vector.tensor_scalar(
                    out=adh, in0=xns[:, 0:T],
                    scalar1=scal[:, OFF_UD + 0 * H + h: OFF_UD + 0 * H + h + 1],
                    scalar2=None, op0=OP.mult)
                for j in range(1, nj_x):
                    nc.vector.scalar_tensor_tensor(
                        out=adh, in0=xns[:, j * T:(j + 1) * T],
                        scalar=scal[:, OFF_UD + j * H + h: OFF_UD + j * H + h + 1],
                        in1=adh, op0=OP.mult, op1=OP.add)
            ad16 = wp.tile([P, H * T], F16, tag="ad16")
            nc.vector.tensor_copy(ad16[:], ad_all[:])

            # pad-slot denominator correction: corr = npad * exp(prelu(ad))
            mcol = wp.tile([P, 1], dt, tag="mcol")
            nc.vector.memset(mcol[:], MSHIFT)
            corr = wp.tile([P, H * T], dt, tag="corr")
            tpr = wp.tile([P, H * T], F16, tag="tpr")
            nc.scalar.activation(tpr[:], ad16[:], AF.Prelu, alpha=NEG_SLOPE)
            nc.scalar.activation(corr[:], tpr[:], AF.Exp, bias=mcol[:, :1])
            npd_b = AP(npd[:].tensor, npd[:].offset,
                       [list(npd[:].ap[0]), [0, H], [1, T]])
            nc.vector.tensor_tensor(out=corr[:].rearrange(
                "p (h t) -> p h t", h=H), in0=corr[:].rearrange(
                "p (h t) -> p h t", h=H), in1=npd_b, op=OP.mult)

            # persistent per-chunk accumulators + output staging
            den_g = [wp.tile([P, H * CH], dt, tag=f"den{g}", name=f"den{g}")
                     for g in range(NG)]
            agg_g = [wp.tile([P, H * nj_x * CH], dt, tag=f"agg{g}",
                             name=f"agg{g}") for g in range(NG)]
            out_sb = wp.tile([P, T * lat], dt, tag="out_sb")

            def chunks_of(bt0, bt1):
                return range(bt0 // CH, (bt1 + CH - 1) // CH)

            # ------------- per-block edge pipeline (sw-pipelined) ----------
            # Emission order: trees(0) | [logits-tail(b) ; trees(b+1) ;
            # post-exp(b) ; phase2(b)] so DVE never waits on pool/ACT.
            blk = {}

            def emit_dma(b):
                bt0, bt1 = bl[b]
                o0, o1 = int(off[bt0]), int(off[bt1])
                SB = o1 - o0
                eab = sp.tile([P, nj_e * SBmax], F16, tag="eab")
                xgb = sp.tile([P, nj_x * SBmax], F16, tag="xgb")
                aev = sp.tile([P, H * SBmax], F16, tag="aev")
                exb = sp.tile([P, H * SBmax], F16, tag="exb")
                scr = sp.tile([P, 6 * SBmax], F16, tag="scr")
                blk[b] = (o0, SB, eab, xgb, aev, exb, scr)
                for j in range(nj_e):
                    nc.sync.dma_start(eab[:, j * SB:(j + 1) * SB],
                                      ea7_d[:, j * S + o0: j * S + o1])
                for j in range(nj_x):
                    nc.sync.dma_start(xgb[:, j * SB:(j + 1) * SB],
                                      xgv_d[:, j * S + o0: j * S + o1])

            def emit_trees(b):
                bt0, bt1 = bl[b]
                o0, SB, eab, xgb, aev, exb, scr = blk[b]
                ae_t, ae_o = aev[:].tensor, aev[:].offset
                ae_p = list(aev[:].ap[0])
                sc = lambda k: scr[:, k * SB:(k + 1) * SB]

                # V-tree per head: ae_v[h] = sum_j ea_j * V[j,h]
                for h in range(H):
                    aeh = aev[:, h * SB:(h + 1) * SB]
                    nc.vector.tensor_scalar(
                        out=aeh, in0=eab[:, 0:SB],
                        scalar1=scal[:, OFF_V + h: OFF_V + h + 1],
                        scalar2=None, op0=OP.mult)
                    for j in range(1, nj_e):
                        nc.vector.tensor_scalar(
                            out=sc(j - 1), in0=eab[:, j * SB:(j + 1) * SB],
                            scalar1=scal[:, OFF_V + j * H + h:
                                         OFF_V + j * H + h + 1],
                            scalar2=None, op0=OP.mult)
                    nc.vector.tensor_tensor(out=aeh, in0=aeh, in1=sc(0),
                                            op=OP.add)
                    nc.vector.tensor_tensor(out=sc(1), in0=sc(1), in1=sc(2),
                                            op=OP.add)
                    nc.vector.tensor_tensor(out=sc(3), in0=sc(3), in1=sc(4),
                                            op=OP.add)
                    nc.vector.tensor_tensor(out=sc(3), in0=sc(3), in1=sc(5),
                                            op=OP.add)
                    nc.vector.tensor_tensor(out=sc(1), in0=sc(1), in1=sc(3),
                                            op=OP.add)
                    nc.vector.tensor_tensor(out=aeh, in0=aeh, in1=sc(1),
                                            op=OP.add)

                # self-loop logit = mean of real-edge V-logits (per chunk)
                for g in chunks_of(bt0, bt1):
                    ta, tb = g * CH, min((g + 1) * CH, T)
                    tcn = tb - ta
                    dt_g = int(D[ta])
                    lt = int(off[ta]) - o0
                    aes = mp.tile([P, CH * H], dt, tag="aes")
                    nc.vector.tensor_reduce(
                        out=AP(aes[:].tensor, aes[:].offset,
                               [list(aes[:].ap[0]), [1, tcn], [CH, H]]),
                        in_=AP(ae_t, ae_o + lt,
                               [ae_p, [dt_g, tcn], [SB, H], [1, dt_g]]),
                        axis=mybir.AxisListType.X, op=OP.add)
                    ivd_b = AP(ivd[:].tensor, ivd[:].offset + ta,
                               [list(ivd[:].ap[0]), [1, tcn], [0, H]])
                    aes_b = AP(aes[:].tensor, aes[:].offset,
                               [list(aes[:].ap[0]), [1, tcn], [CH, H]])
                    nc.vector.tensor_tensor(
                        out=AP(ae_t, ae_o + lt,
                               [ae_p, [dt_g, tcn], [SB, H]]),
                        in0=aes_b, in1=ivd_b, op=OP.mult)

                # U-tree per head: ae += sum_j xg_j * U_src[j,h]
                for h in range(H):
                    aeh = aev[:, h * SB:(h + 1) * SB]
                    for j in range(nj_x):
                        nc.vector.tensor_scalar(
                            out=sc(j), in0=xgb[:, j * SB:(j + 1) * SB],
                            scalar1=scal[:, OFF_US + j * H + h:
                                         OFF_US + j * H + h + 1],
                            scalar2=None, op0=OP.mult)
                    nc.vector.tensor_tensor(out=sc(0), in0=sc(0), in1=sc(1),
                                            op=OP.add)
                    nc.vector.tensor_tensor(out=sc(0), in0=sc(0), in1=sc(2),
                                            op=OP.add)
                    nc.vector.tensor_tensor(out=aeh, in0=aeh, in1=sc(0),
                                            op=OP.add)

            def emit_logits_tail(b):
                """a_dst add (GpSimd) + leaky-relu + exp (ACT)."""
                bt0, bt1 = bl[b]
                o0, SB, eab, xgb, aev, exb, scr = blk[b]
                ae_t, ae_o = aev[:].tensor, aev[:].offset
                ae_p = list(aev[:].ap[0])
                for g in chunks_of(bt0, bt1):
                    ta, tb = g * CH, min((g + 1) * CH, T)
                    tcn = tb - ta
                    dt_g = int(D[ta])
                    lt = int(off[ta]) - o0
                    sl = AP(ae_t, ae_o + lt,
                            [ae_p, [dt_g, tcn], [SB, H], [1, dt_g]])
                    adb = AP(ad16[:].tensor, ad16[:].offset + ta,
                             [list(ad16[:].ap[0]), [1, tcn], [T, H], [0, dt_g]])
                    nc.gpsimd.tensor_tensor(out=sl, in0=sl, in1=adb, op=OP.add)
                nc.scalar.activation(aev[:, :H * SB], aev[:, :H * SB],
                                     AF.Prelu, alpha=NEG_SLOPE)
                nc.scalar.activation(exb[:, :H * SB], aev[:, :H * SB], AF.Exp,
                                     bias=mcol[:, :1])

            def emit_post(b):
                """denominators, weighted messages, aggregation (per chunk)."""
                bt0, bt1 = bl[b]
                o0, SB, eab, xgb, aev, exb, scr = blk[b]
                ex_t, ex_o = exb[:].tensor, exb[:].offset
                ex_p = list(exb[:].ap[0])
                for g in chunks_of(bt0, bt1):
                    ta, tb = g * CH, min((g + 1) * CH, T)
                    tcn = tb - ta
                    dt_g = int(D[ta])
                    lt = int(off[ta]) - o0
                    dg = den_g[g]
                    ag = agg_g[g]

                    nc.vector.tensor_reduce(
                        out=AP(dg[:].tensor, dg[:].offset,
                               [list(dg[:].ap[0]), [1, tcn], [CH, H]]),
                        in_=AP(ex_t, ex_o + lt,
                               [ex_p, [dt_g, tcn], [SB, H], [1, dt_g]]),
                        axis=mybir.AxisListType.X, op=OP.add)
                    # den -= npad * exp(prelu(a_dst))   (pad-slot correction)
                    cor_b = AP(corr[:].tensor, corr[:].offset + ta,
                               [list(corr[:].ap[0]), [1, tcn], [T, H]])
                    dg_b = AP(dg[:].tensor, dg[:].offset,
                              [list(dg[:].ap[0]), [1, tcn], [CH, H]])
                    nc.vector.tensor_tensor(out=dg_b, in0=dg_b, in1=cor_b,
                                            op=OP.subtract)

                    msg = mp.tile([P, H * nj_x * CH * Dmax], F16, tag="msg")
                    m_t, m_o, m_p = (msg[:].tensor, msg[:].offset,
                                     list(msg[:].ap[0]))
                    nd = nj_x * dt_g
                    for ti in range(ta, tb):
                        lt_i = int(off[ta]) - o0 + (ti - ta) * dt_g
                        m_ap = AP(m_t, m_o + (ti - ta) * H * nd,
                                  [m_p, [nd, H], [dt_g, nj_x], [1, dt_g]])
                        ealpha = AP(ex_t, ex_o + lt_i,
                                    [ex_p, [SB, H], [0, nj_x], [1, dt_g]])
                        xgs = AP(xgb[:].tensor, xgb[:].offset + lt_i,
                                 [list(xgb[:].ap[0]), [0, H], [SB, nj_x],
                                  [1, dt_g]])
                        nc.vector.tensor_tensor(out=m_ap, in0=ealpha, in1=xgs,
                                                op=OP.mult)
                    nc.vector.tensor_reduce(
                        out=AP(ag[:].tensor, ag[:].offset,
                               [list(ag[:].ap[0]), [1, tcn], [CH, H * nj_x]]),
                        in_=AP(m_t, m_o,
                               [m_p, [H * nd, tcn], [dt_g, H * nj_x],
                                [1, dt_g]]),
                        axis=mybir.AxisListType.X, op=OP.add)

            def emit_phase2(b):
                bt0, bt1 = bl[b]
                for g in chunks_of(bt0, bt1):
                    ta, tb = g * CH, min((g + 1) * CH, T)
                    tcn = tb - ta
                    cw = tcn * P
                    dg = den_g[g]
                    ag = agg_g[g]

                    # alpha-normalize: agg_n = agg / den (fp32)
                    rec = mp.tile([P, H * CH], dt, tag="rec")
                    nc.vector.reciprocal(rec[:], dg[:])
                    agn = mp.tile([P, H * nj_x * CH], dt, tag="agn")
                    ag_b = AP(ag[:].tensor, ag[:].offset,
                              [list(ag[:].ap[0]), [CH * nj_x, H], [CH, nj_x],
                               [1, tcn]])
                    an_b = AP(agn[:].tensor, agn[:].offset,
                              [list(agn[:].ap[0]), [CH * nj_x, H], [CH, nj_x],
                               [1, tcn]])
                    rec_b = AP(rec[:].tensor, rec[:].offset,
                               [list(rec[:].ap[0]), [CH, H], [0, nj_x],
                                [1, tcn]])
                    nc.vector.tensor_tensor(out=an_b, in0=ag_b, in1=rec_b,
                                            op=OP.mult)

                    # transpose agg_n -> [12, cw] then to fp16 for matmul
                    pst = pq.tile([nj_x * H, CW], dt, tag="pst")
                    for ti in range(ta, tb):
                        nc.tensor.transpose(
                            out=pst[:, (ti - ta) * P:(ti - ta + 1) * P],
                            in_=AP(agn[:].tensor, agn[:].offset + (ti - ta),
                                   [list(agn[:].ap[0]), [CH, nj_x * H]]),
                            identity=ident[:])
                    aggT = mp.tile([nj_x * H, CW], F16, tag="aggT")
                    nc.scalar.copy(aggT[:, :cw], pst[:, :cw])

                    ps1 = pp.tile([HC, CW], dt, tag="ps1")
                    nc.tensor.matmul(ps1[:, :cw], wpj[:], aggT[:, :cw],
                                     start=True, stop=True)
                    # ELU(z+bg): min(exp(z+bg),1) - 1 + relu(z+bg)
                    r1 = mp.tile([HC, CW], F16, tag="r1")
                    u1 = mp.tile([HC, CW], F16, tag="u1")
                    nc.scalar.activation(r1[:, :cw], ps1[:, :cw], AF.Relu,
                                         bias=bgc[:, :1])
                    nc.scalar.activation(u1[:, :cw], ps1[:, :cw], AF.Exp,
                                         bias=bgc[:, :1])
                    nc.vector.tensor_scalar(out=u1[:, :cw], in0=u1[:, :cw],
                                            scalar1=1.0, scalar2=-1.0,
                                            op0=OP.min, op1=OP.add)
                    nc.vector.tensor_tensor(out=r1[:, :cw], in0=r1[:, :cw],
                                            in1=u1[:, :cw], op=OP.add)

                    ps2 = pp.tile([HC, CW], dt, tag="ps2")
                    nc.tensor.matmul(ps2[:, :cw], w1s[:], r1[:, :cw],
                                     start=True, stop=True)
                    h2 = mp.tile([HC, CW], F16, tag="h2")
                    nc.scalar.activation(h2[:, :cw], ps2[:, :cw], AF.Prelu,
                                         bias=b1c[:, :1], alpha=prelu_alpha)

                    ps3 = pp.tile([lat, CW], dt, tag="ps3")
                    nc.tensor.matmul(ps3[:, :cw], w2s[:], h2[:, :cw],
                                     start=True, stop=True)
                    o3 = mp.tile([lat, CW], dt, tag="o3")
                    nc.scalar.copy(o3[:, :cw], ps3[:, :cw])

                    pso = pq.tile([P, CH * lat], dt, tag="pso")
                    for ti in range(ta, tb):
                        nc.tensor.transpose(
                            out=pso[:, (ti - ta) * lat:(ti - ta + 1) * lat],
                            in_=o3[:, (ti - ta) * P:(ti - ta + 1) * P],
                            identity=ident[:lat, :lat])
                    b2b = AP(b2r[:].tensor, b2r[:].offset,
                             [list(b2r[:].ap[0]), [0, tcn], [1, lat]])
                    nc.vector.scalar_tensor_tensor(
                        out=out_sb[:, ta * lat: tb * lat],
                        in0=pso[:, :tcn * lat],
                        scalar=1.0, in1=b2b, op0=OP.mult, op1=OP.add)
                    nc.sync.dma_start(out_d[:, ta * lat: tb * lat],
                                      out_sb[:, ta * lat: tb * lat])

            NB = len(bl)
            for b in range(NB):
                emit_dma(b)
            emit_trees(0)
            for b in range(NB):
                emit_logits_tail(b)
                if b + 1 < NB:
                    emit_trees(b + 1)
                emit_post(b)
                emit_phase2(b)

    return nc


# ---------------------------------------------------------------------------
# Full kernel entry (host orchestration).
# ---------------------------------------------------------------------------
def make_in_maps(sched, streams, w, n_cores):
    maps = []
    for c in range(n_cores):
        m = dict(
            ea7=streams["ea7"][c].reshape(P, -1),
            xgv=streams["xgv"][c].reshape(P, -1),
            xn3=streams["xn3"][c].reshape(P, -1),
            invd=streams["invd"][c],
            npad=streams["npad"][c],
        )
        m.update(w)
        maps.append(m)
    return maps


def unscramble(results, sched, unscr, N, lat=32):
    n_cores = sched["n_cores"]
    T = sched["T"]
    out = np.zeros((N, lat), dtype=np.float32)
    for c in range(n_cores):
        o = results[c]["out"].reshape(P, T, lat)
        node_of = unscr["node_of"][c]  # [T, P] global ids (clamped for dummies)
        valid = unscr["valid_loc"][c].reshape(T, P)
        for t in range(T):
            v = valid[t]
            out[node_of[t][v]] = o[v, t]
    return out


# ---------------------------------------------------------------------------
# Self-contained harness entry: kernel(**inputs) -> full [N, 32] output.
# ---------------------------------------------------------------------------
_CACHE = {}


def kernel(x, edge_index, edge_attr, W_gat, att_src, att_dst, W_edge,
           att_edge, bias_gat, W1, b1, prelu_a, W2, b2):
    from concourse.bass_utils import run_bass_kernel_spmd

    patch_tile_epilogue()
    n_cores = 8
    x = np.asarray(x)
    edge_index = np.asarray(edge_index)
    edge_attr = np.asarray(edge_attr)
    H, C = np.asarray(att_src).shape

    sched, streams, unscr = host_prep(x, edge_index, edge_attr, n_cores)
    w = host_weights(H, C, np.asarray(W_gat), np.asarray(att_src),
                     np.asarray(att_dst), np.asarray(W_edge),
                     np.asarray(att_edge), np.asarray(bias_gat),
                     np.asarray(W1), np.asarray(b1), np.asarray(prelu_a),
                     np.asarray(W2), np.asarray(b2))

    key = (sched["T"], sched["S"], tuple(int(d) for d in sched["D"]),
           float(np.asarray(prelu_a)))
    if key not in _CACHE:
        _CACHE[key] = build_program(sched, n_heads=H,
                                    prelu_alpha=float(np.asarray(prelu_a)))
    nc = _CACHE[key]

    maps = make_in_maps(sched, streams, w, n_cores)
    res = run_bass_kernel_spmd(nc, maps, core_ids=list(range(n_cores)))
    out = unscramble(res.results, sched, unscr, x.shape[0])
    return out.astype(np.float32)
